# revision 44
# baseline (speedup 1.0000x reference)
"""Trainium2 Bass kernel for a small single-head transformer block.

Math (see reference):
  q,k per-token dot product reduces to a bilinear form:
      scores = x^T (Wq^T Wk / sqrt(D)) x  -> one GEMM (u = x @ M^T) + rowwise dot
  so the block is 4 token-parallel GEMMs (u, v, W1, W2) + masked softmax over
  L=5 + 2 layernorms, all batch-parallel across 8 cores.

Layout: samples on SBUF partitions, (l, d) in the free dim, so softmax(L) and
LN(D) are free-axis ops.  GEMM operands need the contraction dim (d) on
partitions; x arrives pre-transposed from the host, n1/h are transposed on the
tensor engine.  All matmuls run in fp8-e4m3 with perf_mode=DoubleRow (2 fp8
weights per PE cell -> K=256 per matmul, ~1.5x bf16 FLOP/s) and fp32 PSUM
accumulation.  fp8 range handling: M = Wq^T Wk/sqrt(D) has sigma ~4e-4, far
below e4m3's 2^-9 denormal floor, so it is pre-scaled by 2^10 (descaled via
the keep mask); Wv/W1 are scaled by 2^5 (descaled for free in the ACT-engine
eviction/gelu `scale`).  Per-feature GEMM biases (only when nonzero) are
injected into PSUM via a K=2 ones-row matmul carrying a hi/lo fp16 split of
the bias.

The per-tile emission is phase-shifted (tile i+1's u/v GEMMs are emitted
before tile i's epilogue) so the in-order PE queue never waits on the DVE
softmax/LN chain — otherwise the PE idles ~13.6us per tile and the HAM
clock-gate re-throttles it cold every tile.
"""

import numpy as np

B, L, D = 16384, 5, 1024
NCORES = 8
BLOC = B // NCORES          # samples per core
P = 128                     # samples per tile
NTILES = BLOC // P
KT = D // 128               # contraction k-tiles
KP = KT // 2                # DoubleRow k-tile pairs
NB = D // 512               # 512-wide PSUM column blocks
LN_EPS = 1e-5
# fp8 range management: every GEMM operand is pre-scaled so its sigma sits
# well inside e4m3's normal range (2^-6..240), and every descale is folded
# into an op that already exists (ACT `scale`, the keep mask, or LN2's
# scale invariance).  The residual branch runs at 256x: s_t = 256*(ff+n1),
# which LN2 normalizes away exactly.
SX = 8.0                    # xT = fp8(8x)
SW = 32.0                   # Wv/W1/W2 stored as fp8(32*W)
SU = 1024.0                 # bilinear matrix M stored as fp8(1024*M)
SRES = 256.0                # residual-branch scale (n1g = 256*n1)
SN1T = 32.0                 # n1T = fp8(32*n1)
SH = 8.0                    # hT = fp8(8*h)

_cache = {}


def _hi_lo_f16(v32):
    hi = v32.astype(np.float16)
    lo = (v32 - hi.astype(np.float32)).astype(np.float16)
    return np.stack([hi, lo], 0)  # [2, D]


def _build(apply_att_affine, apply_ff_affine, has_bias_h, has_bias_s,
           keep_pat, ntiles=NTILES):
    kept = [l for l in range(L) if keep_pat[l]]
    masked = [l for l in range(L) if not keep_pat[l]]
    import concourse.bacc as bacc
    import concourse.mybir as mybir
    from concourse.tile import TileContext
    from concourse.masks import make_identity
    from contextlib import ExitStack

    f16 = mybir.dt.float16
    f32 = mybir.dt.float32
    f8 = mybir.dt.float8e4
    DR = mybir.MatmulPerfMode.DoubleRow
    AF = mybir.ActivationFunctionType
    OP = mybir.AluOpType
    AX = mybir.AxisListType

    nc = bacc.Bacc("TRN2", target_bir_lowering=False, debug=False,
                   num_devices=NCORES)

    # ---- DRAM I/O ----
    xbf = nc.dram_tensor("xbf", [BLOC, L * D], f16, kind="ExternalInput")
    xT = nc.dram_tensor("xT", [KT, 128, NTILES, L * P], f8,
                        kind="ExternalInput")
    wts = {
        n: nc.dram_tensor(n, [KT, 128, D], f8, kind="ExternalInput")
        for n in ("wu", "wv", "w1", "w2")
    }
    if has_bias_h:
        bias_h_d = nc.dram_tensor("bias_h", [2, D], f16, kind="ExternalInput")
    if has_bias_s:
        bias_s_d = nc.dram_tensor("bias_s", [2, D], f16, kind="ExternalInput")
    keep_d = nc.dram_tensor("keep", [1, L], f32, kind="ExternalInput")
    mneg_d = nc.dram_tensor("mneg", [1, L], f32, kind="ExternalInput")
    if masked:
        # masked tokens have z = x, so their LN1 output is a pure per-sample
        # function of the input -- precomputed on the host (O(B*D) prep),
        # both sample-major (residual path) and pre-transposed fp8 (GEMM
        # operand, saving 8 PE transposes + an ACT evict per masked token).
        n1m_d = nc.dram_tensor("n1m", [BLOC, len(masked), D], f16,
                               kind="ExternalInput")
        n1mT_d = nc.dram_tensor("n1mT", [KT, 128, NTILES, len(masked) * P],
                                f8, kind="ExternalInput")
    if apply_att_affine:
        attg_d = nc.dram_tensor("attg", [1, D], f16, kind="ExternalInput")
    if apply_ff_affine:
        ffg_d = nc.dram_tensor("ffg", [1, D], f32, kind="ExternalInput")
        ffb_d = nc.dram_tensor("ffb", [1, D], f32, kind="ExternalInput")
    out_d = nc.dram_tensor("out", [BLOC, L, D], f32, kind="ExternalOutput")

    with TileContext(nc) as tc, ExitStack() as ctx:
        const = ctx.enter_context(tc.tile_pool(name="const", bufs=1))
        px = ctx.enter_context(tc.tile_pool(name="px", bufs=3))
        pxT = ctx.enter_context(tc.tile_pool(name="pxT", bufs=3))
        pvh = ctx.enter_context(tc.tile_pool(name="pvh", bufs=3))
        pzh = ctx.enter_context(tc.tile_pool(name="pzh", bufs=2))
        pns = ctx.enter_context(tc.tile_pool(name="pns", bufs=3))
        pn1 = ctx.enter_context(tc.tile_pool(name="pn1", bufs=2))
        psm = ctx.enter_context(tc.tile_pool(name="psm", bufs=3))
        pout = ctx.enter_context(tc.tile_pool(name="pout", bufs=2))
        pps = ctx.enter_context(tc.tile_pool(name="pps", bufs=8, space="PSUM"))

        # ---- constants / weights (resident) ----
        # wu/wv loads are emitted inside emit_uv(0) per-k (after tile 0's
        # x/xT DMAs); w1/w2 ride with emit_uv(1).
        w_sb = {n: const.tile([128, KT, D], f8, tag=n, name=n)
                for n in ("wu", "wv", "w1", "w2")}
        if has_bias_h:
            bias_h = const.tile([2, D], f16, tag="bias_h")
            nc.sync.dma_start(out=bias_h, in_=bias_h_d[:, :])
        if has_bias_s:
            bias_s = const.tile([2, D], f16, tag="bias_s")
            nc.sync.dma_start(out=bias_s, in_=bias_s_d[:, :])
        if has_bias_h or has_bias_s:
            ones2 = const.tile([2, 128], f16, tag="ones2")
            nc.vector.memset(ones2, 1.0)
        ident = const.tile([128, 128], f16, tag="ident")
        make_identity(nc, ident)
        keep_b = const.tile([128, L], f32, tag="keep")
        nc.gpsimd.dma_start(out=keep_b, in_=keep_d[:, :].to_broadcast([128, L]))
        mneg_b = const.tile([128, L], f32, tag="mneg")
        nc.gpsimd.dma_start(out=mneg_b, in_=mneg_d[:, :].to_broadcast([128, L]))
        if apply_att_affine:
            attg_b = const.tile([128, D], f16, tag="attg")
            nc.gpsimd.dma_start(out=attg_b,
                                in_=attg_d[:, :].to_broadcast([128, D]))
        if apply_ff_affine:
            ffg_b = const.tile([128, D], f32, tag="ffg")
            nc.gpsimd.dma_start(out=ffg_b,
                                in_=ffg_d[:, :].to_broadcast([128, D]))
            ffb_b = const.tile([128, D], f32, tag="ffb")
            nc.gpsimd.dma_start(out=ffb_b,
                                in_=ffb_d[:, :].to_broadcast([128, D]))

        state = {}

        def emit_uv(i):
            """DMA x/xT for tile i, u&v GEMMs (fp8 DoubleRow), raw scores."""
            x_t = px.tile([128, L, D], f16, tag="x")
            nc.sync.dma_start(out=x_t,
                              in_=xbf[i * P:(i + 1) * P, :].rearrange(
                                  "p (l d) -> p l d", l=L))
            if i == 0:
                wnames = ("wu", "wv") + (("w1", "w2") if ntiles == 1 else ())
            elif i == 1:
                wnames = ("w1", "w2")
            else:
                wnames = ()
            xT_t = pxT.tile([128, KT, L * P], f8, tag="xT")
            for k in range(KT):
                nc.sync.dma_start(out=xT_t[:, k, :], in_=xT[k, :, i, :])
                for n in wnames:
                    nc.sync.dma_start(out=w_sb[n][:, k, :],
                                      in_=wts[n][k, :, :])

            v_sb = pvh.tile([128, L, D], f16, tag="vh")
            sc2 = psm.tile([128, L, NB], f32, tag="sc2")
            nc.vector.memset(sc2, 0.0)  # masked l slots stay 0

            for l in kept:
                lhs = [xT_t[:, 2 * k:2 * k + 2, l * P:(l + 1) * P]
                       for k in range(KP)]
                for half, wname in ((0, "wu"), (1, "wv")):
                    for nb in range(NB):
                        ps = pps.tile([128, 512], f32, tag="mm")
                        for k in range(KP):
                            nc.tensor.matmul(
                                ps, lhs[k],
                                w_sb[wname][:, 2 * k:2 * k + 2,
                                            nb * 512:(nb + 1) * 512],
                                start=(k == 0), stop=(k == KP - 1),
                                perf_mode=DR)
                        if half == 0:  # u -> scores partials, fused mul+reduce
                            scr = psm.tile([128, 512], f16, tag="scr")
                            nc.vector.scalar_tensor_tensor(
                                out=scr, in0=ps, scalar=1.0,
                                in1=x_t[:, l, nb * 512:(nb + 1) * 512],
                                op0=OP.mult, op1=OP.mult,
                                accum_out=sc2[:, l, nb:nb + 1])
                        else:  # v -> SBUF (descale the x*Wv fp8 pre-scales)
                            nc.scalar.activation(
                                out=v_sb[:, l, nb * 512:(nb + 1) * 512],
                                in_=ps, func=AF.Copy, scale=1.0 / (SX * SW))
            state[i] = (x_t, v_sb, sc2)

        def _newton_rsqrt(v_t, y0, tags):
            """y = v^-0.5 via 4 Newton steps from constant guess y0.

            Valid when v*y0^2 is within ~[0.35, 2.9] (convergence), which
            the LN variances here satisfy by construction.  Runs entirely
            on DVE smalls so ACT never leaves the gelu table set.
            """
            ta, tb, tsq, tw = tags
            ys = [psm.tile([128, L], f32, tag=ta, name=ta),
                  psm.tile([128, L], f32, tag=tb, name=tb)]
            sq = psm.tile([128, L], f32, tag=tsq, name=tsq)
            w_t = psm.tile([128, L], f32, tag=tw, name=tw)
            nc.vector.memset(ys[0], y0)
            cur = 0
            for _ in range(4):
                src, dst = ys[cur], ys[1 - cur]
                nc.vector.tensor_mul(sq, src, src)
                nc.vector.scalar_tensor_tensor(
                    out=w_t, in0=sq, scalar=-0.5, in1=v_t,
                    op0=OP.mult, op1=OP.mult)
                nc.vector.scalar_tensor_tensor(
                    out=dst, in0=w_t, scalar=1.5, in1=src,
                    op0=OP.add, op1=OP.mult)
                cur = 1 - cur
            return ys[cur]

        def emit_zchain(i):
            """Softmax, z, LN1 stats + Newton, n1 for tile i (DVE/ACT only).

            Emitted one tile ahead of emit_ffn(i) so n1(i) is ready well
            before the PE reaches tile i's transposes."""
            x_t, v_sb, sc2 = state[i]

            # ---- masked softmax over L; exp(x)=(1+t)/(1-t), t=tanh(x/2),
            # so ACT stays in the gelu table set (no exp/ln table loads) ----
            ssum = psm.tile([128, L], f32, tag="ssum")
            nc.vector.tensor_add(ssum, sc2[:, :, 0], sc2[:, :, 1])
            scm = psm.tile([128, L], f32, tag="scm")
            nc.vector.tensor_mul(scm, ssum, keep_b)
            nc.vector.tensor_add(scm, scm, mneg_b)
            th = psm.tile([128, L], f32, tag="th")
            nc.scalar.activation(th, scm, AF.Tanh, scale=0.5)
            a_t = psm.tile([128, L], f32, tag="a")
            nc.vector.tensor_scalar(out=a_t, in0=th, scalar1=-1.0,
                                    scalar2=1.0, op0=OP.mult, op1=OP.add)
            r_t = psm.tile([128, L], f32, tag="r")
            nc.vector.reciprocal(r_t, a_t)
            e_t = psm.tile([128, L], f32, tag="e")
            nc.vector.scalar_tensor_tensor(out=e_t, in0=th, scalar=1.0,
                                           in1=r_t, op0=OP.add, op1=OP.mult)
            den = psm.tile([128, 1], f32, tag="den")
            nc.vector.reduce_sum(den, e_t, axis=AX.X)
            rden = psm.tile([128, 1], f32, tag="rden")
            nc.vector.reciprocal(rden, den)
            probs = psm.tile([128, L], f32, tag="probs")
            nc.vector.tensor_scalar_mul(probs, e_t, rden)

            # ---- z = probs*v + x, fused on DVE with free running-sum (the
            # accum gives Sum(z)); Sum(z^2) via one fused square per kept l.
            # Masked l need nothing here: their LN1 output n1m arrives
            # pre-computed from the host straight into the n1 tile. ----
            n1 = pn1.tile([128, L, D], f16, tag="n1")
            for j, ml in enumerate(masked):
                nc.sync.dma_start(out=n1[:, ml, :],
                                  in_=n1m_d[i * P:(i + 1) * P, j, :])
            z_t = pzh.tile([128, L, D], f16, tag="zhT")
            zs1 = psm.tile([128, L], f32, tag="zs1")
            nc.vector.memset(zs1, 1.0)   # masked lanes: benign stats
            zq1 = psm.tile([128, L], f32, tag="zq1")
            nc.vector.memset(zq1, 2.0)
            for l in kept:
                nc.vector.scalar_tensor_tensor(
                    out=z_t[:, l, :], in0=v_sb[:, l, :],
                    scalar=probs[:, l:l + 1], in1=x_t[:, l, :],
                    op0=OP.mult, op1=OP.add,
                    accum_out=zs1[:, l:l + 1])
                zqs = psm.tile([128, D], f16, tag="zqs")
                nc.vector.scalar_tensor_tensor(
                    out=zqs, in0=z_t[:, l, :], scalar=1.0,
                    in1=z_t[:, l, :], op0=OP.mult, op1=OP.mult,
                    accum_out=zq1[:, l:l + 1])

            # mean/var smalls + rstd1 = 256*(var+eps)^-0.5 via Newton
            mu1 = psm.tile([128, L], f32, tag="mu1")
            nc.vector.tensor_scalar(out=mu1, in0=zs1, scalar1=1.0 / D,
                                    scalar2=None, op0=OP.mult)
            v1 = psm.tile([128, L], f32, tag="v1")
            nc.vector.tensor_mul(v1, mu1, mu1)
            nc.vector.scalar_tensor_tensor(
                out=v1, in0=zq1, scalar=1.0 / D, in1=v1,
                op0=OP.mult, op1=OP.subtract)
            nc.vector.tensor_scalar(out=v1, in0=v1,
                                    scalar1=LN_EPS, scalar2=2.0 ** -16,
                                    op0=OP.add, op1=OP.mult)
            y1 = _newton_rsqrt(v1, SRES, ("y1a", "y1b", "y1s", "y1w"))

            # ---- n1 = (z - mu)*rstd1 on DVE (256-scaled f16), kept l ----
            for l in kept:
                nc.vector.tensor_scalar(
                    out=n1[:, l, :], in0=z_t[:, l, :],
                    scalar1=mu1[:, l:l + 1], scalar2=y1[:, l:l + 1],
                    op0=OP.subtract, op1=OP.mult)
            state[i] = (n1,)

        def emit_ffn(i):
            """Transposes + W1/gelu + W2 GEMMs for tile i (PE/ACT)."""
            (n1,) = state[i]

            # ---- n1T: kept l transposed on PE (evict on ACT: x1/8 ->
            # fp8(32*n1)); masked l DMA'd pre-transposed from the host ----
            n1T = pns.tile([128, KT, L * P], f8, tag="n1Ts")
            for j, ml in enumerate(masked):
                for k in range(KT):
                    nc.sync.dma_start(
                        out=n1T[:, k, ml * P:(ml + 1) * P],
                        in_=n1mT_d[k, :, i, j * P:(j + 1) * P])
            for l in kept:
                tp = pps.tile([128, KT * 128], f16, tag="mm")
                for k in range(KT):
                    nc.tensor.transpose(
                        tp[:, k * 128:(k + 1) * 128],
                        n1[:, l, k * 128:(k + 1) * 128], ident)
                nc.scalar.activation(
                    out=n1T[:, :, l * P:(l + 1) * P],
                    in_=tp.rearrange("p (a b) -> p a b", a=KT),
                    func=AF.Copy, scale=float(SN1T / SRES))

            # ---- h = gelu((32n1)@(32W1)/1024) ----
            h_sb = pvh.tile([128, L, D], f16, tag="vh")
            for l in range(L):
                for nb in range(NB):
                    ps = pps.tile([128, 512], f32, tag="mm")
                    if has_bias_h:
                        nc.tensor.matmul(ps, ones2,
                                         bias_h[:, nb * 512:(nb + 1) * 512],
                                         start=True, stop=False)
                    for k in range(KP):
                        nc.tensor.matmul(
                            ps, n1T[:, 2 * k:2 * k + 2, l * P:(l + 1) * P],
                            w_sb["w1"][:, 2 * k:2 * k + 2,
                                       nb * 512:(nb + 1) * 512],
                            start=(k == 0 and not has_bias_h),
                            stop=(k == KP - 1), perf_mode=DR)
                    nc.scalar.activation(
                        out=h_sb[:, l, nb * 512:(nb + 1) * 512],
                        in_=ps, func=AF.Gelu, scale=1.0 / (SN1T * SW))

            # ---- transpose h -> hT (evict on ACT: x8 -> fp8(8h)) ----
            hT = pzh.tile([128, KT, L * P], f8, tag="zhT")
            for l in range(L):
                tp = pps.tile([128, KT * 128], f16, tag="mm")
                for k in range(KT):
                    nc.tensor.transpose(
                        tp[:, k * 128:(k + 1) * 128],
                        h_sb[:, l, k * 128:(k + 1) * 128], ident)
                nc.scalar.activation(
                    out=hT[:, :, l * P:(l + 1) * P],
                    in_=tp.rearrange("p (a b) -> p a b", a=KT),
                    func=AF.Copy, scale=float(SH))

            # ---- ff' = (8h)@(32W2) = 256ff; s' = ff' + 256n1 fused on DVE
            # right behind each PSUM (frees the bank; Sum(s') via accum).
            # Only 8 PSUM banks exist, so the adds cannot move to the tail
            # phase -- holding 10 W2 outputs across phases would deadlock
            # the pool ring. ----
            if apply_att_affine:
                n1g = pn1.tile([128, L, D], f16, tag="n1g")
                nc.gpsimd.tensor_mul(n1g, n1, attg_b)
            else:
                n1g = n1
            s_t = pns.tile([128, L, D], f16, tag="n1Ts")
            sm2 = psm.tile([128, L, NB], f32, tag="sm2")
            for l in range(L):
                for nb in range(NB):
                    ps = pps.tile([128, 512], f32, tag="mm")
                    if has_bias_s:
                        nc.tensor.matmul(ps, ones2,
                                         bias_s[:, nb * 512:(nb + 1) * 512],
                                         start=True, stop=False)
                    for k in range(KP):
                        nc.tensor.matmul(
                            ps, hT[:, 2 * k:2 * k + 2, l * P:(l + 1) * P],
                            w_sb["w2"][:, 2 * k:2 * k + 2,
                                       nb * 512:(nb + 1) * 512],
                            start=(k == 0 and not has_bias_s),
                            stop=(k == KP - 1), perf_mode=DR)
                    nc.vector.scalar_tensor_tensor(
                        out=s_t[:, l, nb * 512:(nb + 1) * 512], in0=ps,
                        scalar=0.0, in1=n1g[:, l, nb * 512:(nb + 1) * 512],
                        op0=OP.add, op1=OP.add,
                        accum_out=sm2[:, l, nb:nb + 1])
            state[i] = (s_t, sm2)

        def emit_tail(i):
            """LN2 squares + Newton + normalize + DMA out for tile i."""
            s_t, sm2 = state.pop(i)
            sq2 = psm.tile([128, L, NB], f32, tag="sq2")
            for l in range(L):
                for nb in range(NB):
                    sqs = psm.tile([128, 512], f16, tag="sqs")
                    if l % 2 == 0 and nb == 0:
                        # balance: 3 of 10 squares on ACT (Square is in the
                        # gelu table set; accum gives Sum((s/16)^2))
                        nc.scalar.activation(
                            out=sqs, in_=s_t[:, l, nb * 512:(nb + 1) * 512],
                            func=AF.Square, scale=2.0 ** -4,
                            accum_out=sq2[:, l, nb:nb + 1])
                    else:
                        nc.vector.scalar_tensor_tensor(
                            out=sqs, in0=s_t[:, l, nb * 512:(nb + 1) * 512],
                            scalar=2.0 ** -8,
                            in1=s_t[:, l, nb * 512:(nb + 1) * 512],
                            op0=OP.mult, op1=OP.mult,
                            accum_out=sq2[:, l, nb:nb + 1])

            # LN2 aggregation smalls + Newton rstd2 (DVE)
            mu2 = psm.tile([128, L], f32, tag="mu2")
            nc.vector.tensor_add(mu2, sm2[:, :, 0], sm2[:, :, 1])
            nc.vector.tensor_scalar(out=mu2, in0=mu2, scalar1=1.0 / D,
                                    scalar2=None, op0=OP.mult)
            e2t = psm.tile([128, L], f32, tag="e2t")
            nc.vector.tensor_add(e2t, sq2[:, :, 0], sq2[:, :, 1])
            nc.vector.tensor_scalar(out=e2t, in0=e2t, scalar1=256.0 / D,
                                    scalar2=None, op0=OP.mult)
            v2 = psm.tile([128, L], f32, tag="v2")
            nc.vector.tensor_mul(v2, mu2, mu2)
            nc.vector.tensor_sub(v2, e2t, v2)
            nc.vector.tensor_scalar(out=v2, in0=v2,
                                    scalar1=LN_EPS * 65536.0, scalar2=None,
                                    op0=OP.add)
            y2 = _newton_rsqrt(v2, 2.0 ** -8, ("y2a", "y2b", "y2s", "y2w"))

            # ---- out = Identity(s'*rstd2 + (-mu'*rstd2)) on ACT; DMA out ----
            nmr = psm.tile([128, L], f32, tag="nmr")
            nc.vector.tensor_mul(nmr, mu2, y2)
            nc.vector.tensor_scalar(out=nmr, in0=nmr, scalar1=-1.0,
                                    scalar2=None, op0=OP.mult)
            for l in range(L):
                o_t = pout.tile([128, D], f32, tag="o")
                if apply_ff_affine:
                    n2 = psm.tile([128, D], f16, tag="n2")
                    nc.scalar.activation(
                        out=n2, in_=s_t[:, l, :], func=AF.Identity,
                        scale=y2[:, l:l + 1], bias=nmr[:, l:l + 1])
                    nc.vector.tensor_mul(o_t, n2, ffg_b)
                    nc.vector.tensor_add(o_t, o_t, ffb_b)
                else:
                    nc.scalar.activation(
                        out=o_t, in_=s_t[:, l, :], func=AF.Identity,
                        scale=y2[:, l:l + 1], bias=nmr[:, l:l + 1])
                nc.sync.dma_start(out=out_d[i * P:(i + 1) * P, l, :], in_=o_t)

        # Software pipeline, 3 tiles deep with a decoupled epilogue:
        #   uv(i+2) | zchain(i+1) | ffn(i) | tail(i-1)
        # zchain(i+1) runs a full iteration ahead of its PE consumer
        # (ffn(i+1)), and tail(i-1) is emitted AFTER ffn(i) so its ACT ops
        # (squares + out-normalize) queue behind ffn(i)'s evicts/gelu in the
        # strict-FIFO ACT engine -- otherwise they head-of-line block the
        # PE<->ACT ping-pong and the HAM clock gate re-throttles every tile.
        emit_uv(0)
        if ntiles > 1:
            emit_uv(1)
        emit_zchain(0)
        for i in range(ntiles):
            if i + 2 < ntiles:
                emit_uv(i + 2)
            if i + 1 < ntiles:
                emit_zchain(i + 1)
            emit_ffn(i)
            if i >= 1:
                emit_tail(i - 1)
        emit_tail(ntiles - 1)

    nc.compile()
    return nc


def _prep(x, mask, Wq, Wk, Wv, W1, b1, W2, b2, att_g, att_b, ff_g, ff_b):
    """Host-side preprocessing -> (flags, per-core input maps)."""
    import ml_dtypes
    F8 = ml_dtypes.float8_e4m3  # the TRN FP8_EXP4 variant (max 240)

    f64 = np.float64
    assert SH * SW == SRES and SN1T * SRES == SX * SU
    apply_att_affine = not (np.all(att_g == 1.0) and np.all(att_b == 0.0))
    apply_ff_affine = not (np.all(ff_g == 1.0) and np.all(ff_b == 0.0))

    M = (Wq.astype(f64).T @ Wk.astype(f64)) / np.sqrt(np.float64(D))
    wu = np.ascontiguousarray(M.T * SU).astype(F8)             # [d', d]
    wv = np.ascontiguousarray(Wv.T * SW).astype(F8)            # [d, e]
    W1g = W1.astype(f64) * att_g.astype(f64)[None, :]
    w1 = np.ascontiguousarray(W1g.T * SW).astype(F8)           # [d, e]
    bias_h_f = (SN1T * SW) * (
        b1.astype(f64) + W1.astype(f64) @ att_b.astype(f64)).astype(
        np.float32)
    w2 = np.ascontiguousarray(W2.T * SW).astype(F8)            # [e, f]
    bias_s_f = SRES * (b2.astype(f64) + att_b.astype(f64)).astype(np.float32)
    has_bias_h = bool(np.any(bias_h_f != 0.0))
    has_bias_s = bool(np.any(bias_s_f != 0.0))

    keep = (np.all(mask != 0, axis=0)).astype(np.float32)[None, :]  # [1, L]
    keep_pat = tuple(bool(k) for k in keep[0])
    mneg = (keep - 1.0) * 30.0
    keep = keep / (SX * SU)    # descale raw scores inside the mask multiply

    # masked tokens: z = x, so LN1(z) is a pure per-sample function of the
    # input; precompute it host-side (O(B*D)) at the 256x residual scale.
    masked_idx = [l for l in range(L) if not keep_pat[l]]
    n1m = n1mT = None
    if masked_idx:
        xm = x[:, masked_idx, :].astype(np.float32)
        mu = xm.mean(-1, keepdims=True, dtype=np.float64).astype(np.float32)
        var = xm.var(-1, keepdims=True, dtype=np.float64).astype(np.float32)
        n1mf = (xm - mu) / np.sqrt(var + LN_EPS)
        n1m = (n1mf * SRES).astype(np.float16)
        n1mT = (n1mf * SN1T).astype(F8)         # fp8(32*n1), pre-transpose

    def wfmt(w):  # [D, D] -> [KT, 128, D]
        return np.ascontiguousarray(w.reshape(KT, 128, D))

    shared = dict(
        wu=wfmt(wu), wv=wfmt(wv), w1=wfmt(w1), w2=wfmt(w2),
        keep=keep, mneg=mneg)
    if has_bias_h:
        shared["bias_h"] = _hi_lo_f16(bias_h_f)
    if has_bias_s:
        shared["bias_s"] = _hi_lo_f16(bias_s_f)
    if apply_att_affine:
        shared["attg"] = att_g.astype(np.float16)[None, :]
    if apply_ff_affine:
        shared["ffg"] = ff_g.astype(np.float32)[None, :]
        shared["ffb"] = ff_b.astype(np.float32)[None, :]

    x16 = x.astype(np.float16)
    x8 = (x * SX).astype(F8)
    in_maps = []
    for c in range(NCORES):
        xc = x16[c * BLOC:(c + 1) * BLOC]                      # [BLOC, L, D]
        xbf = np.ascontiguousarray(xc.reshape(BLOC, L * D))
        # [i, s, l, k, dk] -> [k, dk, i, l, s]
        xTc = np.ascontiguousarray(
            x8[c * BLOC:(c + 1) * BLOC]
            .reshape(NTILES, P, L, KT, 128).transpose(3, 4, 0, 2, 1)
        ).reshape(KT, 128, NTILES, L * P)
        m = dict(shared, xbf=xbf, xT=xTc)
        if n1m is not None:
            m["n1m"] = np.ascontiguousarray(n1m[c * BLOC:(c + 1) * BLOC])
            # [i, s, j, k, dk] -> [k, dk, i, j, s]
            m["n1mT"] = np.ascontiguousarray(
                n1mT[c * BLOC:(c + 1) * BLOC]
                .reshape(NTILES, P, len(masked_idx), KT, 128)
                .transpose(3, 4, 0, 2, 1)
            ).reshape(KT, 128, NTILES, len(masked_idx) * P)
        in_maps.append(m)
    flags = (apply_att_affine, apply_ff_affine, has_bias_h, has_bias_s,
             keep_pat)
    return flags, in_maps


def kernel(**inputs):
    from concourse.bass_utils import run_bass_kernel_spmd

    inputs = {k: np.asarray(v) for k, v in inputs.items()}
    flags, in_maps = _prep(**inputs)
    if flags not in _cache:
        _cache[flags] = _build(*flags)
    nc = _cache[flags]
    res = run_bass_kernel_spmd(nc, in_maps, core_ids=list(range(NCORES)))
    out = np.concatenate([r["out"] for r in res.results], axis=0)
    return out.astype(np.float32)



# revision 49
# speedup vs baseline: 1.0839x; 1.0839x over previous
"""Trainium2 Bass kernel for a small single-head transformer block.

Math (see reference):
  q,k per-token dot product reduces to a bilinear form:
      scores = x^T (Wq^T Wk / sqrt(D)) x  -> one GEMM (u = x @ M^T) + rowwise dot
  so the block is 4 token-parallel GEMMs (u, v, W1, W2) + masked softmax over
  L=5 + 2 layernorms, all batch-parallel across 8 cores.

Layout: samples on SBUF partitions, (l, d) in the free dim, so softmax(L) and
LN(D) are free-axis ops.  GEMM operands need the contraction dim (d) on
partitions; x arrives pre-transposed from the host, n1/h are transposed on the
tensor engine.  All matmuls run in fp8-e4m3 with perf_mode=DoubleRow (2 fp8
weights per PE cell -> K=256 per matmul, ~1.5x bf16 FLOP/s) and fp32 PSUM
accumulation.  fp8 range handling: M = Wq^T Wk/sqrt(D) has sigma ~4e-4, far
below e4m3's 2^-9 denormal floor, so it is pre-scaled by 2^10 (descaled via
the keep mask); Wv/W1 are scaled by 2^5 (descaled for free in the ACT-engine
eviction/gelu `scale`).  Per-feature GEMM biases (only when nonzero) are
injected into PSUM via a K=2 ones-row matmul carrying a hi/lo fp16 split of
the bias.

The per-tile emission is phase-shifted (tile i+1's u/v GEMMs are emitted
before tile i's epilogue) so the in-order PE queue never waits on the DVE
softmax/LN chain — otherwise the PE idles ~13.6us per tile and the HAM
clock-gate re-throttles it cold every tile.
"""

import numpy as np

B, L, D = 16384, 5, 1024
NCORES = 8
BLOC = B // NCORES          # samples per core
P = 128                     # samples per tile
NTILES = BLOC // P
KT = D // 128               # contraction k-tiles
KP = KT // 2                # DoubleRow k-tile pairs
NB = D // 512               # 512-wide PSUM column blocks
LN_EPS = 1e-5
# fp8 range management: every GEMM operand is pre-scaled so its sigma sits
# well inside e4m3's normal range (2^-6..240), and every descale is folded
# into an op that already exists (ACT `scale`, the keep mask, or LN2's
# scale invariance).  The residual branch runs at 256x: s_t = 256*(ff+n1),
# which LN2 normalizes away exactly.
SX = 8.0                    # xT = fp8(8x)
SW = 32.0                   # Wv/W1/W2 stored as fp8(32*W)
SU = 1024.0                 # bilinear matrix M stored as fp8(1024*M)
SRES = 256.0                # residual-branch scale (n1g = 256*n1)
SN1T = 32.0                 # n1T = fp8(32*n1)
SH = 8.0                    # hT = fp8(8*h)

_cache = {}


def _hi_lo_f16(v32):
    hi = v32.astype(np.float16)
    lo = (v32 - hi.astype(np.float32)).astype(np.float16)
    return np.stack([hi, lo], 0)  # [2, D]


def _build(apply_att_affine, apply_ff_affine, has_bias_h, has_bias_s,
           keep_pat, ntiles=NTILES):
    kept = [l for l in range(L) if keep_pat[l]]
    masked = [l for l in range(L) if not keep_pat[l]]
    import concourse.bacc as bacc
    import concourse.mybir as mybir
    from concourse.tile import TileContext
    from concourse.masks import make_identity
    from contextlib import ExitStack

    f16 = mybir.dt.float16
    f32 = mybir.dt.float32
    f8 = mybir.dt.float8e4
    DR = mybir.MatmulPerfMode.DoubleRow
    AF = mybir.ActivationFunctionType
    OP = mybir.AluOpType
    AX = mybir.AxisListType

    nc = bacc.Bacc("TRN2", target_bir_lowering=False, debug=False,
                   num_devices=NCORES)

    # ---- DRAM I/O ----
    xbf = nc.dram_tensor("xbf", [BLOC, L * D], f16, kind="ExternalInput")
    xT = nc.dram_tensor("xT", [KT, 128, NTILES, L * P], f8,
                        kind="ExternalInput")
    wts = {
        n: nc.dram_tensor(n, [KT, 128, D], f8, kind="ExternalInput")
        for n in ("wu", "wv", "w1", "w2")
    }
    if has_bias_h:
        bias_h_d = nc.dram_tensor("bias_h", [2, D], f16, kind="ExternalInput")
    if has_bias_s:
        bias_s_d = nc.dram_tensor("bias_s", [2, D], f16, kind="ExternalInput")
    keep_d = nc.dram_tensor("keep", [1, L], f32, kind="ExternalInput")
    mneg_d = nc.dram_tensor("mneg", [1, L], f32, kind="ExternalInput")
    if masked:
        # masked tokens have z = x, so their LN1 output is a pure per-sample
        # function of the input -- precomputed on the host (O(B*D) prep),
        # both sample-major (residual path) and pre-transposed fp8 (GEMM
        # operand, saving 8 PE transposes + an ACT evict per masked token).
        n1m_d = nc.dram_tensor("n1m", [BLOC, len(masked), D], f16,
                               kind="ExternalInput")
        n1mT_d = nc.dram_tensor("n1mT", [128, KT, NTILES, len(masked) * P],
                                f8, kind="ExternalInput")
        # contiguous runs of masked l (l and j advance together)
        mruns = []
        for j, ml in enumerate(masked):
            if mruns and mruns[-1][0] + mruns[-1][2] == ml:
                mruns[-1][2] += 1
            else:
                mruns.append([ml, j, 1])
    if apply_att_affine:
        attg_d = nc.dram_tensor("attg", [1, D], f16, kind="ExternalInput")
    if apply_ff_affine:
        ffg_d = nc.dram_tensor("ffg", [1, D], f32, kind="ExternalInput")
        ffb_d = nc.dram_tensor("ffb", [1, D], f32, kind="ExternalInput")
    out_d = nc.dram_tensor("out", [BLOC, L, D], f32, kind="ExternalOutput")

    with TileContext(nc) as tc, ExitStack() as ctx:
        const = ctx.enter_context(tc.tile_pool(name="const", bufs=1))
        px = ctx.enter_context(tc.tile_pool(name="px", bufs=3))
        pxT = ctx.enter_context(tc.tile_pool(name="pxT", bufs=3))
        pvh = ctx.enter_context(tc.tile_pool(name="pvh", bufs=3))
        pzh = ctx.enter_context(tc.tile_pool(name="pzh", bufs=2))
        pns = ctx.enter_context(tc.tile_pool(name="pns", bufs=3))
        pn1 = ctx.enter_context(tc.tile_pool(name="pn1", bufs=2))
        psm = ctx.enter_context(tc.tile_pool(name="psm", bufs=3))
        pout = ctx.enter_context(tc.tile_pool(name="pout", bufs=2))
        pps = ctx.enter_context(tc.tile_pool(name="pps", bufs=8, space="PSUM"))

        # ---- constants / weights (resident) ----
        # wu/wv loads are emitted inside emit_uv(0) per-k (after tile 0's
        # x/xT DMAs); w1/w2 ride with emit_uv(1).
        w_sb = {n: const.tile([128, KT, D], f8, tag=n, name=n)
                for n in ("wu", "wv", "w1", "w2")}
        if has_bias_h:
            bias_h = const.tile([2, D], f16, tag="bias_h")
            nc.sync.dma_start(out=bias_h, in_=bias_h_d[:, :])
        if has_bias_s:
            bias_s = const.tile([2, D], f16, tag="bias_s")
            nc.sync.dma_start(out=bias_s, in_=bias_s_d[:, :])
        if has_bias_h or has_bias_s:
            ones2 = const.tile([2, 128], f16, tag="ones2")
            nc.vector.memset(ones2, 1.0)
        ident = const.tile([128, 128], f16, tag="ident")
        make_identity(nc, ident)
        keep_b = const.tile([128, L], f32, tag="keep")
        nc.gpsimd.dma_start(out=keep_b, in_=keep_d[:, :].to_broadcast([128, L]))
        mneg_b = const.tile([128, L], f32, tag="mneg")
        nc.gpsimd.dma_start(out=mneg_b, in_=mneg_d[:, :].to_broadcast([128, L]))
        if apply_att_affine:
            attg_b = const.tile([128, D], f16, tag="attg")
            nc.gpsimd.dma_start(out=attg_b,
                                in_=attg_d[:, :].to_broadcast([128, D]))
        if apply_ff_affine:
            ffg_b = const.tile([128, D], f32, tag="ffg")
            nc.gpsimd.dma_start(out=ffg_b,
                                in_=ffg_d[:, :].to_broadcast([128, D]))
            ffb_b = const.tile([128, D], f32, tag="ffb")
            nc.gpsimd.dma_start(out=ffb_b,
                                in_=ffb_d[:, :].to_broadcast([128, D]))

        state = {}

        def emit_uv(i):
            """DMA x/xT for tile i, u&v GEMMs (fp8 DoubleRow), raw scores."""
            x_t = px.tile([128, L, D], f16, tag="x")
            nc.sync.dma_start(out=x_t,
                              in_=xbf[i * P:(i + 1) * P, :].rearrange(
                                  "p (l d) -> p l d", l=L))
            if i == 0:
                wnames = ("wu", "wv") + (("w1", "w2") if ntiles == 1 else ())
            elif i == 1:
                wnames = ("w1", "w2")
            else:
                wnames = ()
            xT_t = pxT.tile([128, KT, L * P], f8, tag="xT")
            for k in range(KT):
                nc.sync.dma_start(out=xT_t[:, k, :], in_=xT[k, :, i, :])
                for n in wnames:
                    nc.sync.dma_start(out=w_sb[n][:, k, :],
                                      in_=wts[n][k, :, :])

            v_sb = pvh.tile([128, L, D], f16, tag="vh")
            sc2 = psm.tile([128, L, NB], f32, tag="sc2")
            nc.vector.memset(sc2, 0.0)  # masked l slots stay 0

            for l in kept:
                lhs = [xT_t[:, 2 * k:2 * k + 2, l * P:(l + 1) * P]
                       for k in range(KP)]
                for half, wname in ((0, "wu"), (1, "wv")):
                    for nb in range(NB):
                        ps = pps.tile([128, 512], f32, tag="mm")
                        for k in range(KP):
                            nc.tensor.matmul(
                                ps, lhs[k],
                                w_sb[wname][:, 2 * k:2 * k + 2,
                                            nb * 512:(nb + 1) * 512],
                                start=(k == 0), stop=(k == KP - 1),
                                perf_mode=DR)
                        if half == 0:  # u -> scores partials, fused mul+reduce
                            scr = psm.tile([128, 512], f16, tag="scr")
                            nc.vector.scalar_tensor_tensor(
                                out=scr, in0=ps, scalar=1.0,
                                in1=x_t[:, l, nb * 512:(nb + 1) * 512],
                                op0=OP.mult, op1=OP.mult,
                                accum_out=sc2[:, l, nb:nb + 1])
                        else:  # v -> SBUF (descale the x*Wv fp8 pre-scales)
                            nc.scalar.activation(
                                out=v_sb[:, l, nb * 512:(nb + 1) * 512],
                                in_=ps, func=AF.Copy, scale=1.0 / (SX * SW))
            state[i] = (x_t, v_sb, sc2)

        def _newton_rsqrt(v_t, y0, tags):
            """y = v^-0.5 via 4 Newton steps from constant guess y0.

            Valid when v*y0^2 is within ~[0.35, 2.9] (convergence), which
            the LN variances here satisfy by construction.  Runs entirely
            on DVE smalls so ACT never leaves the gelu table set.
            """
            ta, tb, tsq, tw = tags
            ys = [psm.tile([128, L], f32, tag=ta, name=ta),
                  psm.tile([128, L], f32, tag=tb, name=tb)]
            sq = psm.tile([128, L], f32, tag=tsq, name=tsq)
            w_t = psm.tile([128, L], f32, tag=tw, name=tw)
            nc.vector.memset(ys[0], y0)
            cur = 0
            for _ in range(3):
                src, dst = ys[cur], ys[1 - cur]
                nc.vector.tensor_mul(sq, src, src)
                nc.vector.scalar_tensor_tensor(
                    out=w_t, in0=sq, scalar=-0.5, in1=v_t,
                    op0=OP.mult, op1=OP.mult)
                nc.vector.scalar_tensor_tensor(
                    out=dst, in0=w_t, scalar=1.5, in1=src,
                    op0=OP.add, op1=OP.mult)
                cur = 1 - cur
            return ys[cur]

        def emit_zchain(i):
            """Softmax, z, LN1 stats + Newton, n1 for tile i (DVE/ACT only).

            Emitted one tile ahead of emit_ffn(i) so n1(i) is ready well
            before the PE reaches tile i's transposes."""
            x_t, v_sb, sc2 = state[i]

            # ---- masked softmax over L; exp(x)=(1+t)/(1-t), t=tanh(x/2),
            # so ACT stays in the gelu table set (no exp/ln table loads) ----
            ssum = psm.tile([128, L], f32, tag="ssum")
            nc.vector.tensor_add(ssum, sc2[:, :, 0], sc2[:, :, 1])
            scm = psm.tile([128, L], f32, tag="scm")
            nc.vector.tensor_mul(scm, ssum, keep_b)
            nc.vector.tensor_add(scm, scm, mneg_b)
            th = psm.tile([128, L], f32, tag="th")
            nc.scalar.activation(th, scm, AF.Tanh, scale=0.5)
            a_t = psm.tile([128, L], f32, tag="a")
            nc.vector.tensor_scalar(out=a_t, in0=th, scalar1=-1.0,
                                    scalar2=1.0, op0=OP.mult, op1=OP.add)
            r_t = psm.tile([128, L], f32, tag="r")
            nc.vector.reciprocal(r_t, a_t)
            e_t = psm.tile([128, L], f32, tag="e")
            nc.vector.scalar_tensor_tensor(out=e_t, in0=th, scalar=1.0,
                                           in1=r_t, op0=OP.add, op1=OP.mult)
            den = psm.tile([128, 1], f32, tag="den")
            nc.vector.reduce_sum(den, e_t, axis=AX.X)
            rden = psm.tile([128, 1], f32, tag="rden")
            nc.vector.reciprocal(rden, den)
            probs = psm.tile([128, L], f32, tag="probs")
            nc.vector.tensor_scalar_mul(probs, e_t, rden)

            # ---- z = probs*v + x, fused on DVE with free running-sum (the
            # accum gives Sum(z)); Sum(z^2) via one fused square per kept l.
            # Masked l need nothing here: their LN1 output n1m arrives
            # pre-computed from the host straight into the n1 tile. ----
            n1 = pn1.tile([128, L, D], f16, tag="n1")
            for l0, j0, ln in mruns:
                nc.gpsimd.dma_start(
                    out=n1[:, l0:l0 + ln, :],
                    in_=n1m_d[i * P:(i + 1) * P, j0:j0 + ln, :])
            z_t = pzh.tile([128, L, D], f16, tag="zhT")
            zs1 = psm.tile([128, L], f32, tag="zs1")
            nc.vector.memset(zs1, 1.0)   # masked lanes: benign stats
            zq1 = psm.tile([128, L], f32, tag="zq1")
            nc.vector.memset(zq1, 2.0)
            for l in kept:
                nc.vector.scalar_tensor_tensor(
                    out=z_t[:, l, :], in0=v_sb[:, l, :],
                    scalar=probs[:, l:l + 1], in1=x_t[:, l, :],
                    op0=OP.mult, op1=OP.add,
                    accum_out=zs1[:, l:l + 1])
                zqs = psm.tile([128, D], f16, tag="zqs")
                nc.vector.scalar_tensor_tensor(
                    out=zqs, in0=z_t[:, l, :], scalar=1.0,
                    in1=z_t[:, l, :], op0=OP.mult, op1=OP.mult,
                    accum_out=zq1[:, l:l + 1])

            # mean/var smalls + rstd1 = 256*(var+eps)^-0.5 via Newton
            mu1 = psm.tile([128, L], f32, tag="mu1")
            nc.vector.tensor_scalar(out=mu1, in0=zs1, scalar1=1.0 / D,
                                    scalar2=None, op0=OP.mult)
            v1 = psm.tile([128, L], f32, tag="v1")
            nc.vector.tensor_mul(v1, mu1, mu1)
            nc.vector.scalar_tensor_tensor(
                out=v1, in0=zq1, scalar=1.0 / D, in1=v1,
                op0=OP.mult, op1=OP.subtract)
            nc.vector.tensor_scalar(out=v1, in0=v1,
                                    scalar1=LN_EPS, scalar2=2.0 ** -16,
                                    op0=OP.add, op1=OP.mult)
            y1 = _newton_rsqrt(v1, SRES, ("y1a", "y1b", "y1s", "y1w"))

            # ---- n1 = (z - mu)*rstd1 on DVE (256-scaled f16), kept l ----
            for l in kept:
                nc.vector.tensor_scalar(
                    out=n1[:, l, :], in0=z_t[:, l, :],
                    scalar1=mu1[:, l:l + 1], scalar2=y1[:, l:l + 1],
                    op0=OP.subtract, op1=OP.mult)
            state[i] = (n1,)

        def emit_ffn(i):
            """Transposes + W1/gelu + W2 GEMMs for tile i (PE/ACT)."""
            (n1,) = state[i]

            # ---- n1T: kept l transposed on PE (evict on ACT: x1/8 ->
            # fp8(32*n1)); masked l DMA'd pre-transposed from the host ----
            n1T = pns.tile([128, KT, L * P], f8, tag="n1Ts")
            for l0, j0, ln in mruns:
                nc.gpsimd.dma_start(
                    out=n1T[:, :, l0 * P:(l0 + ln) * P],
                    in_=n1mT_d[:, :, i, j0 * P:(j0 + ln) * P])
            for l in kept:
                tp = pps.tile([128, KT * 128], f16, tag="mm")
                for k in range(KT):
                    nc.tensor.transpose(
                        tp[:, k * 128:(k + 1) * 128],
                        n1[:, l, k * 128:(k + 1) * 128], ident)
                nc.scalar.activation(
                    out=n1T[:, :, l * P:(l + 1) * P],
                    in_=tp.rearrange("p (a b) -> p a b", a=KT),
                    func=AF.Copy, scale=float(SN1T / SRES))

            # ---- h = gelu((32n1)@(32W1)/1024) ----
            h_sb = pvh.tile([128, L, D], f16, tag="vh")
            for l in range(L):
                for nb in range(NB):
                    ps = pps.tile([128, 512], f32, tag="mm")
                    if has_bias_h:
                        nc.tensor.matmul(ps, ones2,
                                         bias_h[:, nb * 512:(nb + 1) * 512],
                                         start=True, stop=False)
                    for k in range(KP):
                        nc.tensor.matmul(
                            ps, n1T[:, 2 * k:2 * k + 2, l * P:(l + 1) * P],
                            w_sb["w1"][:, 2 * k:2 * k + 2,
                                       nb * 512:(nb + 1) * 512],
                            start=(k == 0 and not has_bias_h),
                            stop=(k == KP - 1), perf_mode=DR)
                    nc.scalar.activation(
                        out=h_sb[:, l, nb * 512:(nb + 1) * 512],
                        in_=ps, func=AF.Gelu, scale=1.0 / (SN1T * SW))

            # ---- transpose h -> hT (evict on ACT: x8 -> fp8(8h)) ----
            hT = pzh.tile([128, KT, L * P], f8, tag="zhT")
            for l in range(L):
                tp = pps.tile([128, KT * 128], f16, tag="mm")
                for k in range(KT):
                    nc.tensor.transpose(
                        tp[:, k * 128:(k + 1) * 128],
                        h_sb[:, l, k * 128:(k + 1) * 128], ident)
                nc.scalar.activation(
                    out=hT[:, :, l * P:(l + 1) * P],
                    in_=tp.rearrange("p (a b) -> p a b", a=KT),
                    func=AF.Copy, scale=float(SH))

            # ---- ff' = (8h)@(32W2) = 256ff; s' = ff' + 256n1 fused on DVE
            # right behind each PSUM (frees the bank; Sum(s') via accum).
            # Only 8 PSUM banks exist, so the adds cannot move to the tail
            # phase -- holding 10 W2 outputs across phases would deadlock
            # the pool ring. ----
            if apply_att_affine:
                n1g = pn1.tile([128, L, D], f16, tag="n1g")
                nc.gpsimd.tensor_mul(n1g, n1, attg_b)
            else:
                n1g = n1
            s_t = pns.tile([128, L, D], f16, tag="n1Ts")
            sm2 = psm.tile([128, L, NB], f32, tag="sm2")
            for l in range(L):
                for nb in range(NB):
                    ps = pps.tile([128, 512], f32, tag="mm")
                    if has_bias_s:
                        nc.tensor.matmul(ps, ones2,
                                         bias_s[:, nb * 512:(nb + 1) * 512],
                                         start=True, stop=False)
                    for k in range(KP):
                        nc.tensor.matmul(
                            ps, hT[:, 2 * k:2 * k + 2, l * P:(l + 1) * P],
                            w_sb["w2"][:, 2 * k:2 * k + 2,
                                       nb * 512:(nb + 1) * 512],
                            start=(k == 0 and not has_bias_s),
                            stop=(k == KP - 1), perf_mode=DR)
                    nc.vector.scalar_tensor_tensor(
                        out=s_t[:, l, nb * 512:(nb + 1) * 512], in0=ps,
                        scalar=0.0, in1=n1g[:, l, nb * 512:(nb + 1) * 512],
                        op0=OP.add, op1=OP.add,
                        accum_out=sm2[:, l, nb:nb + 1])
            state[i] = (s_t, sm2)

        def emit_tail(i):
            """LN2 squares + Newton + normalize + DMA out for tile i."""
            s_t, sm2 = state.pop(i)
            sq2 = psm.tile([128, L, NB], f32, tag="sq2")
            for l in range(L):
                for nb in range(NB):
                    sqs = psm.tile([128, 512], f16, tag="sqs")
                    if l % 2 == 0 and nb == 0:
                        # balance: 3 of 10 squares on ACT (Square is in the
                        # gelu table set; accum gives Sum((s/16)^2))
                        nc.scalar.activation(
                            out=sqs, in_=s_t[:, l, nb * 512:(nb + 1) * 512],
                            func=AF.Square, scale=2.0 ** -4,
                            accum_out=sq2[:, l, nb:nb + 1])
                    else:
                        nc.vector.scalar_tensor_tensor(
                            out=sqs, in0=s_t[:, l, nb * 512:(nb + 1) * 512],
                            scalar=2.0 ** -8,
                            in1=s_t[:, l, nb * 512:(nb + 1) * 512],
                            op0=OP.mult, op1=OP.mult,
                            accum_out=sq2[:, l, nb:nb + 1])

            # LN2 aggregation smalls + Newton rstd2 (DVE)
            mu2 = psm.tile([128, L], f32, tag="mu2")
            nc.vector.tensor_add(mu2, sm2[:, :, 0], sm2[:, :, 1])
            nc.vector.tensor_scalar(out=mu2, in0=mu2, scalar1=1.0 / D,
                                    scalar2=None, op0=OP.mult)
            e2t = psm.tile([128, L], f32, tag="e2t")
            nc.vector.tensor_add(e2t, sq2[:, :, 0], sq2[:, :, 1])
            nc.vector.tensor_scalar(out=e2t, in0=e2t, scalar1=256.0 / D,
                                    scalar2=None, op0=OP.mult)
            v2 = psm.tile([128, L], f32, tag="v2")
            nc.vector.tensor_mul(v2, mu2, mu2)
            nc.vector.tensor_sub(v2, e2t, v2)
            nc.vector.tensor_scalar(out=v2, in0=v2,
                                    scalar1=LN_EPS * 65536.0, scalar2=None,
                                    op0=OP.add)
            y2 = _newton_rsqrt(v2, 2.0 ** -8, ("y2a", "y2b", "y2s", "y2w"))

            # ---- out = Identity(s'*rstd2 + (-mu'*rstd2)) on ACT; DMA out ----
            nmr = psm.tile([128, L], f32, tag="nmr")
            nc.vector.tensor_mul(nmr, mu2, y2)
            nc.vector.tensor_scalar(out=nmr, in0=nmr, scalar1=-1.0,
                                    scalar2=None, op0=OP.mult)
            for l in range(L):
                o_t = pout.tile([128, D], f32, tag="o")
                if apply_ff_affine:
                    n2 = psm.tile([128, D], f16, tag="n2")
                    nc.scalar.activation(
                        out=n2, in_=s_t[:, l, :], func=AF.Identity,
                        scale=y2[:, l:l + 1], bias=nmr[:, l:l + 1])
                    nc.vector.tensor_mul(o_t, n2, ffg_b)
                    nc.vector.tensor_add(o_t, o_t, ffb_b)
                else:
                    nc.scalar.activation(
                        out=o_t, in_=s_t[:, l, :], func=AF.Identity,
                        scale=y2[:, l:l + 1], bias=nmr[:, l:l + 1])
                nc.sync.dma_start(out=out_d[i * P:(i + 1) * P, l, :], in_=o_t)

        # Software pipeline, 3 tiles deep with a decoupled epilogue:
        #   uv(i+2) | zchain(i+1) | ffn(i) | tail(i-1)
        # zchain(i+1) runs a full iteration ahead of its PE consumer
        # (ffn(i+1)), and tail(i-1) is emitted AFTER ffn(i) so its ACT ops
        # (squares + out-normalize) queue behind ffn(i)'s evicts/gelu in the
        # strict-FIFO ACT engine -- otherwise they head-of-line block the
        # PE<->ACT ping-pong and the HAM clock gate re-throttles every tile.
        emit_uv(0)
        if ntiles > 1:
            emit_uv(1)
        emit_zchain(0)
        for i in range(ntiles):
            if i + 2 < ntiles:
                emit_uv(i + 2)
            if i + 1 < ntiles:
                emit_zchain(i + 1)
            emit_ffn(i)
            if i >= 1:
                emit_tail(i - 1)
        emit_tail(ntiles - 1)

    nc.compile()
    return nc


def _prep(x, mask, Wq, Wk, Wv, W1, b1, W2, b2, att_g, att_b, ff_g, ff_b):
    """Host-side preprocessing -> (flags, per-core input maps)."""
    import ml_dtypes
    F8 = ml_dtypes.float8_e4m3  # the TRN FP8_EXP4 variant (max 240)

    f64 = np.float64
    assert SH * SW == SRES and SN1T * SRES == SX * SU
    apply_att_affine = not (np.all(att_g == 1.0) and np.all(att_b == 0.0))
    apply_ff_affine = not (np.all(ff_g == 1.0) and np.all(ff_b == 0.0))

    M = (Wq.astype(f64).T @ Wk.astype(f64)) / np.sqrt(np.float64(D))
    wu = np.ascontiguousarray(M.T * SU).astype(F8)             # [d', d]
    wv = np.ascontiguousarray(Wv.T * SW).astype(F8)            # [d, e]
    W1g = W1.astype(f64) * att_g.astype(f64)[None, :]
    w1 = np.ascontiguousarray(W1g.T * SW).astype(F8)           # [d, e]
    bias_h_f = (SN1T * SW) * (
        b1.astype(f64) + W1.astype(f64) @ att_b.astype(f64)).astype(
        np.float32)
    w2 = np.ascontiguousarray(W2.T * SW).astype(F8)            # [e, f]
    bias_s_f = SRES * (b2.astype(f64) + att_b.astype(f64)).astype(np.float32)
    has_bias_h = bool(np.any(bias_h_f != 0.0))
    has_bias_s = bool(np.any(bias_s_f != 0.0))

    keep = (np.all(mask != 0, axis=0)).astype(np.float32)[None, :]  # [1, L]
    keep_pat = tuple(bool(k) for k in keep[0])
    mneg = (keep - 1.0) * 30.0
    keep = keep / (SX * SU)    # descale raw scores inside the mask multiply

    # masked tokens: z = x, so LN1(z) is a pure per-sample function of the
    # input; precompute it host-side (O(B*D)) at the 256x residual scale.
    masked_idx = [l for l in range(L) if not keep_pat[l]]
    n1m = n1mT = None
    if masked_idx:
        xm = x[:, masked_idx, :].astype(np.float32)
        mu = xm.mean(-1, keepdims=True, dtype=np.float64).astype(np.float32)
        var = xm.var(-1, keepdims=True, dtype=np.float64).astype(np.float32)
        n1mf = (xm - mu) / np.sqrt(var + LN_EPS)
        n1m = (n1mf * SRES).astype(np.float16)
        n1mT = (n1mf * SN1T).astype(F8)         # fp8(32*n1), pre-transpose

    def wfmt(w):  # [D, D] -> [KT, 128, D]
        return np.ascontiguousarray(w.reshape(KT, 128, D))

    shared = dict(
        wu=wfmt(wu), wv=wfmt(wv), w1=wfmt(w1), w2=wfmt(w2),
        keep=keep, mneg=mneg)
    if has_bias_h:
        shared["bias_h"] = _hi_lo_f16(bias_h_f)
    if has_bias_s:
        shared["bias_s"] = _hi_lo_f16(bias_s_f)
    if apply_att_affine:
        shared["attg"] = att_g.astype(np.float16)[None, :]
    if apply_ff_affine:
        shared["ffg"] = ff_g.astype(np.float32)[None, :]
        shared["ffb"] = ff_b.astype(np.float32)[None, :]

    x16 = x.astype(np.float16)
    x8 = (x * SX).astype(F8)
    in_maps = []
    for c in range(NCORES):
        xc = x16[c * BLOC:(c + 1) * BLOC]                      # [BLOC, L, D]
        xbf = np.ascontiguousarray(xc.reshape(BLOC, L * D))
        # [i, s, l, k, dk] -> [k, dk, i, l, s]
        xTc = np.ascontiguousarray(
            x8[c * BLOC:(c + 1) * BLOC]
            .reshape(NTILES, P, L, KT, 128).transpose(3, 4, 0, 2, 1)
        ).reshape(KT, 128, NTILES, L * P)
        m = dict(shared, xbf=xbf, xT=xTc)
        if n1m is not None:
            m["n1m"] = np.ascontiguousarray(n1m[c * BLOC:(c + 1) * BLOC])
            # [i, s, j, k, dk] -> [dk, k, i, j, s]
            m["n1mT"] = np.ascontiguousarray(
                n1mT[c * BLOC:(c + 1) * BLOC]
                .reshape(NTILES, P, len(masked_idx), KT, 128)
                .transpose(4, 3, 0, 2, 1)
            ).reshape(128, KT, NTILES, len(masked_idx) * P)
        in_maps.append(m)
    flags = (apply_att_affine, apply_ff_affine, has_bias_h, has_bias_s,
             keep_pat)
    return flags, in_maps


def kernel(**inputs):
    from concourse.bass_utils import run_bass_kernel_spmd

    inputs = {k: np.asarray(v) for k, v in inputs.items()}
    flags, in_maps = _prep(**inputs)
    if flags not in _cache:
        _cache[flags] = _build(*flags)
    nc = _cache[flags]
    res = run_bass_kernel_spmd(nc, in_maps, core_ids=list(range(NCORES)))
    out = np.concatenate([r["out"] for r in res.results], axis=0)
    return out.astype(np.float32)

